# revision 27
# baseline (speedup 1.0000x reference)
"""Dense spatial self-attention block (LayerNorm + single-head attention +
residual) for Trainium2, run data-parallel over batch across 8 NeuronCores.

Shapes (hardcoded from the problem spec):
  x: [B=8, H=64, W=64, C=256] fp32 -> out: same shape.
Each core processes one batch element: T = H*W = 4096 tokens, C = 256.

Per-core algorithm (all matmuls in bf16 with fp32 PSUM accumulation):
  h   = (x - mu) * rsqrt(var + eps)                  (gamma/beta folded into W/b)
  H^T = transpose(h)               [C, T]            (PE transpose, bf16)
  Q^T = (gamma*Wq)^T H^T + bq'     [C, T]            K^T likewise
  V   = H (gamma*Wv) + bv'         [T, C] (+ ones column for softmax denom)
  per 512-token block, per 128-key chunk:
    S^T  = K^T_chunk^T . Q^T_block   (PSUM, fp32)
    P^T  = exp(S^T / sqrt(C))        (ScalarE, bf16 out; no max-sub needed,
                                      |logits| <~ 20 for this distribution)
    O   += P^T^T . [V | 1]           (accumulates values and denominator)
  O_norm = O[:, :C] / O[:, C]; Z = transpose(O_norm) used as lhsT for Wo
  out = x + bo + O_norm @ Wo
"""

import numpy as np

import concourse.bass as bass
import concourse.mybir as mybir
import concourse.tile as tile
from concourse.bass_utils import run_bass_kernel_spmd
from concourse.masks import make_identity

F32 = mybir.dt.float32
BF16 = mybir.dt.bfloat16
AF = mybir.ActivationFunctionType
OP = mybir.AluOpType

B, HH, WW, C = 8, 64, 64, 256
T = HH * WW            # 4096 tokens per core
P = 128
CT = C // P            # 2 channel tiles
TT = T // P            # 32 token tiles
NB = T // 512          # 8 free-dim tiles of 512 over tokens
TBLK = 256             # query-block size for attention
NTB = T // TBLK        # 16 query blocks
MS = TBLK // P         # 2 psum m-tiles per query block
JC = T // P            # 32 key chunks (processed in pairs)
JP = JC // 2           # 16 key-chunk pairs
EPS = 1e-5
SCALE = float(C) ** -0.5


MAX_WAITS_PER_INST = 1


def _split_multi_waits(nc: bass.Bass, max_waits: int = MAX_WAITS_PER_INST):
    """This container's walrus rejects instructions carrying more than ~1
    sync-wait ("Too many sync wait commands"). Hoist excess waits onto
    preceding same-engine InstNoOps (waiting earlier is always safe)."""
    n_split = 0
    for f in nc.m.functions:
        for bb in f.blocks:
            new_insts = []
            for inst in bb.instructions:
                si = getattr(inst, "sync_info", None)
                if si is not None and si.on_wait and len(si.on_wait) > max_waits:
                    waits = list(si.on_wait)
                    keep = waits[-max_waits:]
                    extra = waits[:-max_waits]
                    for i in range(0, len(extra), max_waits):
                        nop = mybir.InstNoOp(
                            name=nc.get_next_instruction_name(), ins=[], outs=[]
                        )
                        nop.engine = inst.engine
                        nop.sync_info = mybir.SyncInfo(
                            on_wait=extra[i : i + max_waits], on_update=[]
                        )
                        nc.register_instruction(nop, overwrite=True)
                        new_insts.append(nop)
                    si.on_wait = keep
                    n_split += 1
                new_insts.append(inst)
            bb.instructions[:] = new_insts
    return n_split


def build(n_reps: int = 1) -> bass.Bass:
    nc = bass.Bass()

    x_d = nc.declare_dram_parameter("x", [T, C], F32, isOutput=False)
    gamma_d = nc.declare_dram_parameter("ln_gamma", [C], F32, isOutput=False)
    beta_d = nc.declare_dram_parameter("ln_beta", [C], F32, isOutput=False)
    wq_d = nc.declare_dram_parameter("wq", [C, C], F32, isOutput=False)
    bq_d = nc.declare_dram_parameter("bq", [C], F32, isOutput=False)
    wk_d = nc.declare_dram_parameter("wk", [C, C], F32, isOutput=False)
    bk_d = nc.declare_dram_parameter("bk", [C], F32, isOutput=False)
    wv_d = nc.declare_dram_parameter("wv", [C, C], F32, isOutput=False)
    bv_d = nc.declare_dram_parameter("bv", [C], F32, isOutput=False)
    wo_d = nc.declare_dram_parameter("wo", [C, C], F32, isOutput=False)
    bo_d = nc.declare_dram_parameter("bo", [C], F32, isOutput=False)
    out_d = nc.declare_dram_parameter("out", [T, C], F32, isOutput=True)

    x_tiled = x_d.rearrange("(o p) c -> p o c", p=P)      # [128, 32, 256]
    out_tiled = out_d.rearrange("(o p) c -> p o c", p=P)  # [128, 32, 256]

    with tile.TileContext(nc) as tc:
        _body(tc, nc, x_tiled, out_tiled, gamma_d, beta_d,
              wq_d, bq_d, wk_d, bk_d, wv_d, bv_d, wo_d, bo_d, n_reps)
    _split_multi_waits(nc, MAX_WAITS_PER_INST)
    return nc


def _body(tc, nc, x_tiled, out_tiled, gamma_d, beta_d,
          wq_d, bq_d, wk_d, bk_d, wv_d, bv_d, wo_d, bo_d, n_reps):
    from contextlib import ExitStack

    ctx = ExitStack()
    singles = ctx.enter_context(tc.tile_pool(name="singles", bufs=1))
    temps = ctx.enter_context(tc.tile_pool(name="temps", bufs=3))
    stats_p = ctx.enter_context(tc.tile_pool(name="stats", bufs=4))
    ps_acc = ctx.enter_context(tc.tile_pool(name="ps_acc", bufs=2, space="PSUM"))
    ps_tp = ctx.enter_context(tc.tile_pool(name="ps_tp", bufs=2, space="PSUM"))
    ps_o = ctx.enter_context(tc.tile_pool(name="ps_o", bufs=2, space="PSUM"))

    def acc_tile(name):
        # all accumulator psum tiles share one tag/footprint (1 bank)
        t = ps_acc.tile([P, 512], F32, tag="acc", name=name)
        return t

    # ---- constants / weights setup -------------------------------------
    ident = singles.tile([P, P], BF16)
    make_identity(nc, ident)

    gamma_col = singles.tile([P, CT], F32)
    nc.sync.dma_start(out=gamma_col, in_=gamma_d.rearrange("(o p) -> p o", p=P))
    beta_col = singles.tile([P, CT], F32)
    nc.sync.dma_start(out=beta_col, in_=beta_d.rearrange("(o p) -> p o", p=P))
    bq_col = singles.tile([P, CT], F32)
    nc.sync.dma_start(out=bq_col, in_=bq_d.rearrange("(o p) -> p o", p=P))
    bk_col = singles.tile([P, CT], F32)
    nc.sync.dma_start(out=bk_col, in_=bk_d.rearrange("(o p) -> p o", p=P))
    bv_row = singles.tile([1, C], F32)
    nc.sync.dma_start(out=bv_row, in_=bv_d[None, :])
    bo_bcast = singles.tile([P, C], F32)
    nc.sync.dma_start(out=bo_bcast, in_=bo_d[None, :].to_broadcast((P, C)))
    eps_t = singles.tile([P, 1], F32)
    nc.vector.memset(eps_t, EPS)
    # Dummy Ln to trigger the one-time ~2.7us natural_log_exp table load on
    # ScalarE while the x DMA is still in flight (instead of serializing it
    # into the first LayerNorm rsqrt chain).
    act_warm = singles.tile([P, 1], F32)
    nc.scalar.activation(out=act_warm, in_=eps_t, func=AF.Ln, bias=1.0)

    # ---- big SBUF tensors ----------------------------------------------
    x_sb = singles.tile([P, TT, C], F32)        # x, later x + bo
    ht_sb = singles.tile([P, CT, T], BF16)      # H^T
    qt_sb = singles.tile([P, CT, T], BF16)      # Q^T
    kt_sb = singles.tile([P, CT, T], BF16)      # K^T
    v_sb = singles.tile([P, TT, C + 2], BF16)   # V plus ones column (+pad)

    nc.vector.memset(v_sb[:, :, C : C + 1], 1.0)

    # staged fp32 weights ([c_in_tile, ct, d]); DMAs emitted after the first
    # x chunk so LayerNorm starts as early as possible
    wq_stg = singles.tile([P, CT, C], F32)
    wk_stg = singles.tile([P, CT, C], F32)
    wv_stg = singles.tile([P, CT, C], F32)
    wo_stg = singles.tile([P, CT, C], F32)
    wq_bf = singles.tile([P, CT, C], BF16)
    wk_bf = singles.tile([P, CT, C], BF16)
    wv_bf = singles.tile([P, CT, C], BF16)
    wo_bf = singles.tile([P, CT, C], BF16)
    bias_q = singles.tile([P, CT], F32)
    bias_k = singles.tile([P, CT], F32)
    bv_bcast = singles.tile([P, C], F32)
    ones_row = singles.tile([1, P], F32)
    nc.vector.memset(ones_row, 1.0)

    def emit_weight_dmas():
        nc.sync.dma_start(out=wq_stg, in_=wq_d.rearrange("(o p) d -> p o d", p=P))
        nc.sync.dma_start(out=wk_stg, in_=wk_d.rearrange("(o p) d -> p o d", p=P))
        nc.sync.dma_start(out=wv_stg, in_=wv_d.rearrange("(o p) d -> p o d", p=P))
        nc.sync.dma_start(out=wo_stg, in_=wo_d.rearrange("(o p) d -> p o d", p=P))

    def emit_weight_folds():
        for ct in range(CT):
            nc.vector.tensor_scalar_mul(wq_bf[:, ct], wq_stg[:, ct], gamma_col[:, ct : ct + 1])
            nc.vector.tensor_scalar_mul(wk_bf[:, ct], wk_stg[:, ct], gamma_col[:, ct : ct + 1])
            nc.vector.tensor_scalar_mul(wv_bf[:, ct], wv_stg[:, ct], gamma_col[:, ct : ct + 1])
            nc.vector.tensor_copy(wo_bf[:, ct], wo_stg[:, ct])
        # folded biases: bias_q[d] = bq[d] + sum_c beta[c] Wq[c, d]  (raw W)
        for (w_stg, b_col, b_out) in ((wq_stg, bq_col, bias_q), (wk_stg, bk_col, bias_k)):
            for dt in range(CT):
                psb = acc_tile("psb")
                for ct in range(CT):
                    nc.tensor.matmul(
                        psb[:, :1],
                        lhsT=w_stg[:, ct, dt * P : (dt + 1) * P],
                        rhs=beta_col[:, ct : ct + 1],
                        start=(ct == 0),
                        stop=(ct == CT - 1),
                    )
                nc.vector.tensor_add(b_out[:, dt : dt + 1], psb[:, :1], b_col[:, dt : dt + 1])
        # bias fold for V: bv_eff[e] = bv[e] + sum_c beta[c] Wv[c, e], bcast
        psv = acc_tile("psv")
        for ct in range(CT):
            nc.tensor.matmul(
                psv[:1, :C],
                lhsT=beta_col[:, ct : ct + 1],
                rhs=wv_stg[:, ct, :],
                start=(ct == 0),
                stop=(ct == CT - 1),
            )
        bv_eff = singles.tile([1, C], F32)
        nc.vector.tensor_add(bv_eff, psv[:1, :C], bv_row)
        # broadcast along partitions via ones-vector matmul (SBUF->SBUF DMA
        # cannot have a zero partition step)
        psb2 = acc_tile("psb2")
        nc.tensor.matmul(psb2[:, :C], lhsT=ones_row, rhs=bv_eff, start=True, stop=True)
        nc.vector.tensor_copy(bv_bcast, psb2[:, :C])

    LNG = 8  # max LN stats batch size (amortizes ACT instruction overhead)
    LN_GROUPS = [4, 4, 8, 8, 8]  # smaller first groups -> earlier first h

    def emit_ln_stats(tt, mv_all, col=None):
        stats = stats_p.tile([P, 6], F32, name="stats")
        nc.vector.bn_stats(out=stats, in_=x_sb[:, tt, :])
        nc.vector.bn_aggr(out=mv_all[:, tt % LNG if col is None else col], in_=stats)

    def emit_ln_rsqrt(mv_all, n=None):
        # rstd = rsqrt(var+eps) = exp(-0.5*ln(var+eps)): keeps every
        # activation in the natural_log_exp_and_others table set (sqrt lives
        # in a different set -> each switch would cost a ~2.7us table load);
        # batched over LNG tiles to amortize the ~300ns ACT fixed cost.
        v = mv_all[:, : (LNG if n is None else n), 1]
        nc.scalar.activation(out=v, in_=v, func=AF.Ln, bias=eps_t)
        nc.scalar.activation(out=v, in_=v, func=AF.Exp, scale=-0.5)

    def emit_ln_apply(tt, mv_all, col):
        xt = x_sb[:, tt, :]
        g = col
        h_bf = temps.tile([P, C], BF16, name="h_bf")
        nc.vector.tensor_scalar(
            out=h_bf, in0=xt,
            scalar1=mv_all[:, g, 0:1], scalar2=mv_all[:, g, 1:2],
            op0=OP.subtract, op1=OP.mult,
        )
        tp = ps_tp.tile([P, CT, P], BF16, tag="tp", name="tp")
        for ct in range(CT):
            nc.tensor.transpose(tp[:, ct], h_bf[:, ct * P : (ct + 1) * P], ident)
        nc.vector.tensor_copy(out=ht_sb[:, :, tt * P : (tt + 1) * P], in_=tp)
        # x_sb <- x + bo (residual including out-proj bias), after LN reads
        nc.gpsimd.tensor_add(out=xt, in0=xt, in1=bo_bcast)

    def emit_qk_proj(ntv):
        for (w_bf, b_t, o_t) in ((wq_bf, bias_q, qt_sb), (wk_bf, bias_k, kt_sb)):
            for dt in range(CT):
                ps = acc_tile("ps")
                for ct in range(CT):
                    nc.tensor.matmul(
                        ps,
                        lhsT=w_bf[:, ct, dt * P : (dt + 1) * P],
                        rhs=ht_sb[:, ct, ntv * 512 : (ntv + 1) * 512],
                        start=(ct == 0),
                        stop=(ct == CT - 1),
                    )
                nc.scalar.activation(
                    out=o_t[:, dt, ntv * 512 : (ntv + 1) * 512],
                    in_=ps,
                    func=AF.Identity,
                    bias=b_t[:, dt : dt + 1],
                )

    def emit_v_pair(jt0):
        psu = acc_tile("psu")
        u2 = psu.rearrange("p (j c) -> p j c", j=2)
        for jj in range(2):
            for ct in range(CT):
                nc.tensor.matmul(
                    u2[:, jj],
                    lhsT=ht_sb[:, ct, (jt0 + jj) * P : (jt0 + jj + 1) * P],
                    rhs=wv_bf[:, ct, :],
                    start=(ct == 0),
                    stop=(ct == CT - 1),
                )
        nc.vector.tensor_tensor(
            out=v_sb[:, jt0 : jt0 + 2, 0:C],
            in0=u2,
            in1=bv_bcast[:, None, :].to_broadcast((P, 2, C)),
            op=OP.add,
        )

    # ---- attention ----------------------------------------------------
    # Two levels of software pipelining (engines execute their streams in
    # order, so emission order IS the PE execution order):
    #  * within a block: S matmuls run one key-pair ahead of the P@V
    #    matmuls so exp(jp) overlaps PE's [PV(jp-1), S(jp+1)] span;
    #  * across blocks: the (normalize, transpose, out-proj, residual)
    #    epilogue of block tb-1 is emitted after block tb's matmul stream,
    #    by which time its DVE inputs are long since ready.
    # Block 0 is additionally woven into the LayerNorm/projection phase
    # (generator driven one key-pair per token tile) to fill PE idle time
    # while DVE works through the LN chains.
    def block_pairs(tb, o_ps):
        pts = [None] * JP
        for jp in range(JP + 1):
            if jp < JP:
                s_ps = acc_tile("s_ps")
                s2 = s_ps.rearrange("p (j t) -> p j t", j=2)
                for jj in range(2):
                    jc = jp * 2 + jj
                    for ct in range(CT):
                        nc.tensor.matmul(
                            s2[:, jj],
                            lhsT=kt_sb[:, ct, jc * P : (jc + 1) * P],
                            rhs=qt_sb[:, ct, tb * TBLK : (tb + 1) * TBLK],
                            start=(ct == 0),
                            stop=(ct == CT - 1),
                        )
                pt = temps.tile([P, 2, TBLK], BF16, name="pt")
                nc.scalar.activation(
                    out=pt.rearrange("p a b -> p (a b)"),
                    in_=s_ps, func=AF.Exp, scale=SCALE,
                )
                pts[jp] = pt
            if jp >= 1:
                pv = jp - 1
                for jj in range(2):
                    jc = pv * 2 + jj
                    for m in range(MS):
                        nc.tensor.matmul(
                            o_ps[:, m, 0 : C + 1],
                            lhsT=pts[pv][:, jj, m * P : (m + 1) * P],
                            rhs=v_sb[:, jc, 0 : C + 1],
                            start=(pv == 0 and jj == 0),
                            stop=(pv == JP - 1 and jj == 1),
                        )
                pts[pv] = None
            yield jp

    def emit_epilogue(tb, o_ps):
        ot_bf = temps.tile([P, CT, TBLK], BF16, name="ot_bf")
        for m in range(MS):
            rec = stats_p.tile([P, 1], F32, name="rec")
            nc.vector.reciprocal(out=rec, in_=o_ps[:, m, C : C + 1])
            onorm = temps.tile([P, C], BF16, name="onorm")
            nc.vector.tensor_scalar_mul(onorm, o_ps[:, m, 0:C], rec)
            tp2 = ps_tp.tile([P, CT, P], BF16, tag="tp", name="tp2")
            for ct in range(CT):
                nc.tensor.transpose(tp2[:, ct], onorm[:, ct * P : (ct + 1) * P], ident)
            nc.vector.tensor_copy(
                out=ot_bf.rearrange("p c (ms pp) -> p c ms pp", ms=MS)[:, :, m],
                in_=tp2,
            )
        for m in range(MS):
            z_ps = ps_tp.tile([P, C], F32, tag="tp", name="z_ps")
            for ct in range(CT):
                nc.tensor.matmul(
                    z_ps,
                    lhsT=ot_bf[:, ct, m * P : (m + 1) * P],
                    rhs=wo_bf[:, ct, :],
                    start=(ct == 0),
                    stop=(ct == CT - 1),
                )
            o_out = temps.tile([P, C], F32, name="o_out")
            gt = tb * MS + m
            nc.vector.tensor_add(o_out, z_ps, x_sb[:, gt, :])
            nc.sync.dma_start(out=out_tiled[:, gt, :], in_=o_out)

    for rep in range(n_reps):
        # ---- load x (first chunk first, then weights, then the rest, so
        # the LayerNorm chain and the weight folds both start early) ------
        nc.sync.dma_start(out=x_sb[:, 0:2, :], in_=x_tiled[:, 0:2, :])
        nc.sync.dma_start(out=x_sb[:, 2:4, :], in_=x_tiled[:, 2:4, :])
        if rep == 0:
            emit_weight_dmas()
        for g in range(1, 8):
            nc.sync.dma_start(
                out=x_sb[:, g * 4 : (g + 1) * 4, :],
                in_=x_tiled[:, g * 4 : (g + 1) * 4, :],
            )

        # LN stats for the first batch of tiles go ahead of the weight-fold
        # work so the DVE starts the moment x arrives and the PE transposes
        # have inputs as early as possible.
        mv_first = stats_p.tile([P, LNG, 2], F32, name="mv_all")
        for i in range(LN_GROUPS[0]):
            emit_ln_stats(i, mv_first, col=i)
        emit_ln_rsqrt(mv_first, LN_GROUPS[0])

        # ---- phase A: layernorm + transposes + projections, with
        # attention blocks 0 and 1 woven in as inputs become available
        # (both only need QT slice ntv=0, ready after tile 3) ------------
        o_tiles = [None] * NTB
        o_tiles[0] = ps_o.tile([P, MS, 512], F32, tag="o", name="o_ps")
        o_tiles[1] = ps_o.tile([P, MS, 512], F32, tag="o", name="o_ps")
        gens = [block_pairs(0, o_tiles[0]), block_pairs(1, o_tiles[1])]
        steps = [0, 0]
        tt_base = 0
        for g, gsz in enumerate(LN_GROUPS):
            if g == 0:
                mv_all = mv_first
            else:
                mv_all = stats_p.tile([P, LNG, 2], F32, name="mv_all")
                for i in range(gsz):
                    emit_ln_stats(tt_base + i, mv_all, col=i)
                emit_ln_rsqrt(mv_all, gsz)
            for i in range(gsz):
                tt = tt_base + i
                emit_ln_apply(tt, mv_all, i)
                if tt == 1 and rep == 0:
                    # must precede the first V pair below, which consumes
                    # wv_bf/bv_bcast (Tile keeps program order; a later
                    # write would NOT be hoisted before this read)
                    emit_weight_folds()
                if tt % 2 == 1:
                    emit_v_pair(tt - 1)
                if tt % 4 == 3:
                    emit_qk_proj(tt // 4)
                if tt >= 4:
                    kt_ready = 2 * ((tt + 1) // 4)
                    allowed = min(JP + 1, kt_ready + (1 if kt_ready >= JP else 0))
                    budget = 2
                    for gi in range(2):
                        while budget > 0 and steps[gi] < allowed:
                            next(gens[gi])
                            steps[gi] += 1
                            budget -= 1
            tt_base += gsz
        for gi in range(2):
            while steps[gi] <= JP:
                next(gens[gi], None)
                steps[gi] += 1

        # ---- remaining attention blocks with pipelined epilogues -------
        for tb in range(2, NTB + 1):
            emit_epilogue(tb - 2, o_tiles[tb - 2])
            o_tiles[tb - 2] = None
            if tb < NTB:
                o_tiles[tb] = ps_o.tile([P, MS, 512], F32, tag="o", name="o_ps")
                for _ in block_pairs(tb, o_tiles[tb]):
                    pass
        emit_epilogue(NTB - 1, o_tiles[NTB - 1])
        o_tiles[NTB - 1] = None

    ctx.close()


_cache = {}


def _get_nc(n_reps: int = 1):
    if n_reps not in _cache:
        _cache[n_reps] = build(n_reps)
    return _cache[n_reps]


def _make_in_maps(inputs):
    x = np.ascontiguousarray(np.asarray(inputs["x"], dtype=np.float32))
    shared = {
        k: np.ascontiguousarray(np.asarray(inputs[k], dtype=np.float32))
        for k in ("ln_gamma", "ln_beta", "wq", "bq", "wk", "bk", "wv", "bv", "wo", "bo")
    }
    return [dict(shared, x=x[i].reshape(T, C)) for i in range(B)]


def kernel(**inputs: np.ndarray) -> np.ndarray:
    nc = _get_nc(1)
    in_maps = _make_in_maps(inputs)
    res = run_bass_kernel_spmd(nc, in_maps, list(range(B)))
    out = np.stack(
        [res.results[i]["out"].reshape(HH, WW, C) for i in range(B)], axis=0
    )
    return out.astype(np.float32)


# revision 28
# speedup vs baseline: 1.1835x; 1.1835x over previous
"""Dense spatial self-attention block (LayerNorm + single-head attention +
residual) for Trainium2, run data-parallel over batch across 8 NeuronCores.

Shapes (hardcoded from the problem spec):
  x: [B=8, H=64, W=64, C=256] fp32 -> out: same shape.
Each core processes one batch element: T = H*W = 4096 tokens, C = 256.

Per-core algorithm (all matmuls in bf16 with fp32 PSUM accumulation):
  h   = (x - mu) * rsqrt(var + eps)                  (gamma/beta folded into W/b)
  H^T = transpose(h)               [C, T]            (PE transpose, bf16)
  Q^T = (gamma*Wq)^T H^T + bq'     [C, T]            K^T likewise
  V   = H (gamma*Wv) + bv'         [T, C] (+ ones column for softmax denom)
  per 512-token block, per 128-key chunk:
    S^T  = K^T_chunk^T . Q^T_block   (PSUM, fp32)
    P^T  = exp(S^T / sqrt(C))        (ScalarE, bf16 out; no max-sub needed,
                                      |logits| <~ 20 for this distribution)
    O   += P^T^T . [V | 1]           (accumulates values and denominator)
  O_norm = O[:, :C] / O[:, C]; Z = transpose(O_norm) used as lhsT for Wo
  out = x + bo + O_norm @ Wo
"""

import numpy as np

import concourse.bass as bass
import concourse.mybir as mybir
import concourse.tile as tile
from concourse.bass_utils import run_bass_kernel_spmd
from concourse.masks import make_identity

F32 = mybir.dt.float32
BF16 = mybir.dt.bfloat16
AF = mybir.ActivationFunctionType
OP = mybir.AluOpType

B, HH, WW, C = 8, 64, 64, 256
T = HH * WW            # 4096 tokens per core
P = 128
CT = C // P            # 2 channel tiles
TT = T // P            # 32 token tiles
NB = T // 512          # 8 free-dim tiles of 512 over tokens
TBLK = 256             # query-block size for attention
NTB = T // TBLK        # 16 query blocks
MS = TBLK // P         # 2 psum m-tiles per query block
JC = T // P            # 32 key chunks (processed in pairs)
JP = JC // 2           # 16 key-chunk pairs
EPS = 1e-5
SCALE = float(C) ** -0.5


MAX_WAITS_PER_INST = 1


def _split_multi_waits(nc: bass.Bass, max_waits: int = MAX_WAITS_PER_INST):
    """This container's walrus rejects instructions carrying more than ~1
    sync-wait ("Too many sync wait commands"). Hoist excess waits onto
    preceding same-engine InstNoOps (waiting earlier is always safe)."""
    n_split = 0
    for f in nc.m.functions:
        for bb in f.blocks:
            new_insts = []
            for inst in bb.instructions:
                si = getattr(inst, "sync_info", None)
                if si is not None and si.on_wait and len(si.on_wait) > max_waits:
                    waits = list(si.on_wait)
                    keep = waits[-max_waits:]
                    extra = waits[:-max_waits]
                    for i in range(0, len(extra), max_waits):
                        nop = mybir.InstNoOp(
                            name=nc.get_next_instruction_name(), ins=[], outs=[]
                        )
                        nop.engine = inst.engine
                        nop.sync_info = mybir.SyncInfo(
                            on_wait=extra[i : i + max_waits], on_update=[]
                        )
                        nc.register_instruction(nop, overwrite=True)
                        new_insts.append(nop)
                    si.on_wait = keep
                    n_split += 1
                new_insts.append(inst)
            bb.instructions[:] = new_insts
    return n_split


def build(n_reps: int = 1) -> bass.Bass:
    nc = bass.Bass()

    x_d = nc.declare_dram_parameter("x", [T, C], F32, isOutput=False)
    gamma_d = nc.declare_dram_parameter("ln_gamma", [C], F32, isOutput=False)
    beta_d = nc.declare_dram_parameter("ln_beta", [C], F32, isOutput=False)
    wq_d = nc.declare_dram_parameter("wq", [C, C], F32, isOutput=False)
    bq_d = nc.declare_dram_parameter("bq", [C], F32, isOutput=False)
    wk_d = nc.declare_dram_parameter("wk", [C, C], F32, isOutput=False)
    bk_d = nc.declare_dram_parameter("bk", [C], F32, isOutput=False)
    wv_d = nc.declare_dram_parameter("wv", [C, C], F32, isOutput=False)
    bv_d = nc.declare_dram_parameter("bv", [C], F32, isOutput=False)
    wo_d = nc.declare_dram_parameter("wo", [C, C], F32, isOutput=False)
    bo_d = nc.declare_dram_parameter("bo", [C], F32, isOutput=False)
    out_d = nc.declare_dram_parameter("out", [T, C], F32, isOutput=True)

    x_tiled = x_d.rearrange("(o p) c -> p o c", p=P)      # [128, 32, 256]
    out_tiled = out_d.rearrange("(o p) c -> p o c", p=P)  # [128, 32, 256]

    with tile.TileContext(nc) as tc:
        _body(tc, nc, x_tiled, out_tiled, gamma_d, beta_d,
              wq_d, bq_d, wk_d, bk_d, wv_d, bv_d, wo_d, bo_d, n_reps)
    _split_multi_waits(nc, MAX_WAITS_PER_INST)
    return nc


def _body(tc, nc, x_tiled, out_tiled, gamma_d, beta_d,
          wq_d, bq_d, wk_d, bk_d, wv_d, bv_d, wo_d, bo_d, n_reps):
    from contextlib import ExitStack

    ctx = ExitStack()
    singles = ctx.enter_context(tc.tile_pool(name="singles", bufs=1))
    temps = ctx.enter_context(tc.tile_pool(name="temps", bufs=3))
    stats_p = ctx.enter_context(tc.tile_pool(name="stats", bufs=4))
    ps_acc = ctx.enter_context(tc.tile_pool(name="ps_acc", bufs=2, space="PSUM"))
    ps_tp = ctx.enter_context(tc.tile_pool(name="ps_tp", bufs=2, space="PSUM"))
    ps_o = ctx.enter_context(tc.tile_pool(name="ps_o", bufs=2, space="PSUM"))

    def acc_tile(name):
        # all accumulator psum tiles share one tag/footprint (1 bank)
        t = ps_acc.tile([P, 512], F32, tag="acc", name=name)
        return t

    # ---- constants / weights setup -------------------------------------
    ident = singles.tile([P, P], BF16)
    make_identity(nc, ident)

    gamma_col = singles.tile([P, CT], F32)
    nc.sync.dma_start(out=gamma_col, in_=gamma_d.rearrange("(o p) -> p o", p=P))
    beta_col = singles.tile([P, CT], F32)
    nc.sync.dma_start(out=beta_col, in_=beta_d.rearrange("(o p) -> p o", p=P))
    bq_col = singles.tile([P, CT], F32)
    nc.sync.dma_start(out=bq_col, in_=bq_d.rearrange("(o p) -> p o", p=P))
    bk_col = singles.tile([P, CT], F32)
    nc.sync.dma_start(out=bk_col, in_=bk_d.rearrange("(o p) -> p o", p=P))
    bv_row = singles.tile([1, C], F32)
    nc.sync.dma_start(out=bv_row, in_=bv_d[None, :])
    bo_bcast = singles.tile([P, C], F32)
    nc.sync.dma_start(out=bo_bcast, in_=bo_d[None, :].to_broadcast((P, C)))
    eps_t = singles.tile([P, 1], F32)
    nc.vector.memset(eps_t, EPS)
    # Dummy Ln to trigger the one-time ~2.7us natural_log_exp table load on
    # ScalarE while the x DMA is still in flight (instead of serializing it
    # into the first LayerNorm rsqrt chain).
    act_warm = singles.tile([P, 1], F32)
    nc.scalar.activation(out=act_warm, in_=eps_t, func=AF.Ln, bias=1.0)

    # ---- big SBUF tensors ----------------------------------------------
    x_sb = singles.tile([P, TT, C], F32)        # x, later x + bo
    ht_sb = singles.tile([P, CT, T], BF16)      # H^T
    qt_sb = singles.tile([P, CT, T], BF16)      # Q^T
    kt_sb = singles.tile([P, CT, T], BF16)      # K^T
    v_sb = singles.tile([P, TT, C + 2], BF16)   # V plus ones column (+pad)

    nc.vector.memset(v_sb[:, :, C : C + 1], 1.0)

    # staged fp32 weights ([c_in_tile, ct, d]); DMAs emitted after the first
    # x chunk so LayerNorm starts as early as possible
    wq_stg = singles.tile([P, CT, C], F32)
    wk_stg = singles.tile([P, CT, C], F32)
    wv_stg = singles.tile([P, CT, C], F32)
    wo_stg = singles.tile([P, CT, C], F32)
    wq_bf = singles.tile([P, CT, C], BF16)
    wk_bf = singles.tile([P, CT, C], BF16)
    wv_bf = singles.tile([P, CT, C], BF16)
    wo_bf = singles.tile([P, CT, C], BF16)
    bias_q = singles.tile([P, CT], F32)
    bias_k = singles.tile([P, CT], F32)
    bv_bcast = singles.tile([P, C], F32)
    ones_row = singles.tile([1, P], F32)
    nc.vector.memset(ones_row, 1.0)

    def emit_weight_dmas():
        nc.sync.dma_start(out=wq_stg, in_=wq_d.rearrange("(o p) d -> p o d", p=P))
        nc.sync.dma_start(out=wk_stg, in_=wk_d.rearrange("(o p) d -> p o d", p=P))
        nc.sync.dma_start(out=wv_stg, in_=wv_d.rearrange("(o p) d -> p o d", p=P))
        nc.sync.dma_start(out=wo_stg, in_=wo_d.rearrange("(o p) d -> p o d", p=P))

    def emit_weight_folds():
        for ct in range(CT):
            nc.vector.tensor_scalar_mul(wq_bf[:, ct], wq_stg[:, ct], gamma_col[:, ct : ct + 1])
            nc.vector.tensor_scalar_mul(wk_bf[:, ct], wk_stg[:, ct], gamma_col[:, ct : ct + 1])
            nc.vector.tensor_scalar_mul(wv_bf[:, ct], wv_stg[:, ct], gamma_col[:, ct : ct + 1])
            nc.vector.tensor_copy(wo_bf[:, ct], wo_stg[:, ct])
        # folded biases: bias_q[d] = bq[d] + sum_c beta[c] Wq[c, d]  (raw W)
        for (w_stg, b_col, b_out) in ((wq_stg, bq_col, bias_q), (wk_stg, bk_col, bias_k)):
            for dt in range(CT):
                psb = acc_tile("psb")
                for ct in range(CT):
                    nc.tensor.matmul(
                        psb[:, :1],
                        lhsT=w_stg[:, ct, dt * P : (dt + 1) * P],
                        rhs=beta_col[:, ct : ct + 1],
                        start=(ct == 0),
                        stop=(ct == CT - 1),
                    )
                nc.vector.tensor_add(b_out[:, dt : dt + 1], psb[:, :1], b_col[:, dt : dt + 1])
        # bias fold for V: bv_eff[e] = bv[e] + sum_c beta[c] Wv[c, e], bcast
        psv = acc_tile("psv")
        for ct in range(CT):
            nc.tensor.matmul(
                psv[:1, :C],
                lhsT=beta_col[:, ct : ct + 1],
                rhs=wv_stg[:, ct, :],
                start=(ct == 0),
                stop=(ct == CT - 1),
            )
        bv_eff = singles.tile([1, C], F32)
        nc.vector.tensor_add(bv_eff, psv[:1, :C], bv_row)
        # broadcast along partitions via ones-vector matmul (SBUF->SBUF DMA
        # cannot have a zero partition step)
        psb2 = acc_tile("psb2")
        nc.tensor.matmul(psb2[:, :C], lhsT=ones_row, rhs=bv_eff, start=True, stop=True)
        nc.vector.tensor_copy(bv_bcast, psb2[:, :C])

    LNG = 8  # max LN stats batch size (amortizes ACT instruction overhead)
    LN_GROUPS = [4, 4, 8, 8, 8]  # smaller first groups -> earlier first h

    def emit_ln_stats(tt, mv_all, col=None):
        stats = stats_p.tile([P, 6], F32, name="stats")
        nc.vector.bn_stats(out=stats, in_=x_sb[:, tt, :])
        nc.vector.bn_aggr(out=mv_all[:, tt % LNG if col is None else col], in_=stats)

    def emit_ln_rsqrt(mv_all, n=None):
        # rstd = rsqrt(var+eps) = exp(-0.5*ln(var+eps)): keeps every
        # activation in the natural_log_exp_and_others table set (sqrt lives
        # in a different set -> each switch would cost a ~2.7us table load);
        # batched over LNG tiles to amortize the ~300ns ACT fixed cost.
        v = mv_all[:, : (LNG if n is None else n), 1]
        nc.scalar.activation(out=v, in_=v, func=AF.Ln, bias=eps_t)
        nc.scalar.activation(out=v, in_=v, func=AF.Exp, scale=-0.5)

    def emit_ln_apply(tt, mv_all, col):
        xt = x_sb[:, tt, :]
        g = col
        h_bf = temps.tile([P, C], BF16, name="h_bf")
        nc.vector.tensor_scalar(
            out=h_bf, in0=xt,
            scalar1=mv_all[:, g, 0:1], scalar2=mv_all[:, g, 1:2],
            op0=OP.subtract, op1=OP.mult,
        )
        tp = ps_tp.tile([P, CT, P], BF16, tag="tp", name="tp")
        for ct in range(CT):
            nc.tensor.transpose(tp[:, ct], h_bf[:, ct * P : (ct + 1) * P], ident)
        nc.vector.tensor_copy(out=ht_sb[:, :, tt * P : (tt + 1) * P], in_=tp)
        # x_sb <- x + bo (residual including out-proj bias), after LN reads
        nc.gpsimd.tensor_add(out=xt, in0=xt, in1=bo_bcast)

    def emit_qk_proj(ntv):
        for (w_bf, b_t, o_t) in ((wq_bf, bias_q, qt_sb), (wk_bf, bias_k, kt_sb)):
            for dt in range(CT):
                ps = acc_tile("ps")
                for ct in range(CT):
                    nc.tensor.matmul(
                        ps,
                        lhsT=w_bf[:, ct, dt * P : (dt + 1) * P],
                        rhs=ht_sb[:, ct, ntv * 512 : (ntv + 1) * 512],
                        start=(ct == 0),
                        stop=(ct == CT - 1),
                    )
                nc.scalar.activation(
                    out=o_t[:, dt, ntv * 512 : (ntv + 1) * 512],
                    in_=ps,
                    func=AF.Identity,
                    bias=b_t[:, dt : dt + 1],
                )

    def emit_v_pair(jt0):
        psu = acc_tile("psu")
        u2 = psu.rearrange("p (j c) -> p j c", j=2)
        for jj in range(2):
            for ct in range(CT):
                nc.tensor.matmul(
                    u2[:, jj],
                    lhsT=ht_sb[:, ct, (jt0 + jj) * P : (jt0 + jj + 1) * P],
                    rhs=wv_bf[:, ct, :],
                    start=(ct == 0),
                    stop=(ct == CT - 1),
                )
        nc.vector.tensor_tensor(
            out=v_sb[:, jt0 : jt0 + 2, 0:C],
            in0=u2,
            in1=bv_bcast[:, None, :].to_broadcast((P, 2, C)),
            op=OP.add,
        )

    # ---- attention ----------------------------------------------------
    # Two levels of software pipelining (engines execute their streams in
    # order, so emission order IS the PE execution order):
    #  * within a block: S matmuls run one key-pair ahead of the P@V
    #    matmuls so exp(jp) overlaps PE's [PV(jp-1), S(jp+1)] span;
    #  * across blocks: the (normalize, transpose, out-proj, residual)
    #    epilogue of block tb-1 is emitted after block tb's matmul stream,
    #    by which time its DVE inputs are long since ready.
    # Block 0 is additionally woven into the LayerNorm/projection phase
    # (generator driven one key-pair per token tile) to fill PE idle time
    # while DVE works through the LN chains.
    def block_pairs(tb, o_ps):
        pts = [None] * JP
        for jp in range(JP + 1):
            if jp < JP:
                s_ps = acc_tile("s_ps")
                s2 = s_ps.rearrange("p (j t) -> p j t", j=2)
                for jj in range(2):
                    jc = jp * 2 + jj
                    for ct in range(CT):
                        nc.tensor.matmul(
                            s2[:, jj],
                            lhsT=kt_sb[:, ct, jc * P : (jc + 1) * P],
                            rhs=qt_sb[:, ct, tb * TBLK : (tb + 1) * TBLK],
                            start=(ct == 0),
                            stop=(ct == CT - 1),
                        )
                pt = temps.tile([P, 2, TBLK], BF16, name="pt")
                nc.scalar.activation(
                    out=pt.rearrange("p a b -> p (a b)"),
                    in_=s_ps, func=AF.Exp, scale=SCALE,
                )
                pts[jp] = pt
            if jp >= 1:
                pv = jp - 1
                for jj in range(2):
                    jc = pv * 2 + jj
                    for m in range(MS):
                        nc.tensor.matmul(
                            o_ps[:, m, 0 : C + 1],
                            lhsT=pts[pv][:, jj, m * P : (m + 1) * P],
                            rhs=v_sb[:, jc, 0 : C + 1],
                            start=(pv == 0 and jj == 0),
                            stop=(pv == JP - 1 and jj == 1),
                        )
                pts[pv] = None
            yield jp

    def emit_epilogue(tb, o_ps):
        ot_bf = temps.tile([P, CT, TBLK], BF16, name="ot_bf")
        for m in range(MS):
            rec = stats_p.tile([P, 1], F32, name="rec")
            nc.vector.reciprocal(out=rec, in_=o_ps[:, m, C : C + 1])
            onorm = temps.tile([P, C], BF16, name="onorm")
            nc.vector.tensor_scalar_mul(onorm, o_ps[:, m, 0:C], rec)
            tp2 = ps_tp.tile([P, CT, P], BF16, tag="tp", name="tp2")
            for ct in range(CT):
                nc.tensor.transpose(tp2[:, ct], onorm[:, ct * P : (ct + 1) * P], ident)
            nc.vector.tensor_copy(
                out=ot_bf.rearrange("p c (ms pp) -> p c ms pp", ms=MS)[:, :, m],
                in_=tp2,
            )
        for m in range(MS):
            z_ps = ps_tp.tile([P, C], F32, tag="tp", name="z_ps")
            for ct in range(CT):
                nc.tensor.matmul(
                    z_ps,
                    lhsT=ot_bf[:, ct, m * P : (m + 1) * P],
                    rhs=wo_bf[:, ct, :],
                    start=(ct == 0),
                    stop=(ct == CT - 1),
                )
            o_out = temps.tile([P, C], F32, name="o_out")
            gt = tb * MS + m
            nc.vector.tensor_add(o_out, z_ps, x_sb[:, gt, :])
            nc.sync.dma_start(out=out_tiled[:, gt, :], in_=o_out)

    for rep in range(n_reps):
        # ---- load x (first chunk first, then weights, then the rest, so
        # the LayerNorm chain and the weight folds both start early) ------
        nc.sync.dma_start(out=x_sb[:, 0:2, :], in_=x_tiled[:, 0:2, :])
        nc.sync.dma_start(out=x_sb[:, 2:4, :], in_=x_tiled[:, 2:4, :])
        if rep == 0:
            emit_weight_dmas()
        for g in range(1, 8):
            nc.sync.dma_start(
                out=x_sb[:, g * 4 : (g + 1) * 4, :],
                in_=x_tiled[:, g * 4 : (g + 1) * 4, :],
            )

        # LN stats for the first batch of tiles go ahead of the weight-fold
        # work so the DVE starts the moment x arrives and the PE transposes
        # have inputs as early as possible.
        mv_first = stats_p.tile([P, LNG, 2], F32, name="mv_all")
        for i in range(LN_GROUPS[0]):
            emit_ln_stats(i, mv_first, col=i)
        emit_ln_rsqrt(mv_first, LN_GROUPS[0])

        # ---- phase A: layernorm + transposes + projections, with
        # attention blocks 0 and 1 woven in as inputs become available
        # (both only need QT slice ntv=0, ready after tile 3) ------------
        o_tiles = [None] * NTB
        o_tiles[0] = ps_o.tile([P, MS, 512], F32, tag="o", name="o_ps")
        o_tiles[1] = ps_o.tile([P, MS, 512], F32, tag="o", name="o_ps")
        gens = [block_pairs(0, o_tiles[0]), block_pairs(1, o_tiles[1])]
        steps = [0, 0]
        tt_base = 0
        for g, gsz in enumerate(LN_GROUPS):
            if g == 0:
                mv_all = mv_first
            else:
                mv_all = stats_p.tile([P, LNG, 2], F32, name="mv_all")
                for i in range(gsz):
                    emit_ln_stats(tt_base + i, mv_all, col=i)
                emit_ln_rsqrt(mv_all, gsz)
            for i in range(gsz):
                tt = tt_base + i
                emit_ln_apply(tt, mv_all, i)
                if tt == 1 and rep == 0:
                    # must precede the first V pair below, which consumes
                    # wv_bf/bv_bcast (Tile keeps program order; a later
                    # write would NOT be hoisted before this read)
                    emit_weight_folds()
                if tt % 2 == 1:
                    emit_v_pair(tt - 1)
                if tt % 4 == 3:
                    emit_qk_proj(tt // 4)
                if tt >= 4:
                    kt_ready = 2 * ((tt + 1) // 4)
                    allowed = min(JP + 1, kt_ready + (1 if kt_ready >= JP else 0))
                    budget = 2
                    for gi in range(2):
                        cap = allowed if gi == 0 else min(allowed, JP)
                        while budget > 0 and steps[gi] < cap:
                            next(gens[gi])
                            steps[gi] += 1
                            budget -= 1
            tt_base += gsz
        # finish block 0 fully; hold block 1's final PV flush so the next
        # block's first S pair can slide in front of it (cross-block S-ahead
        # keeps exp hidden under PE work at every block boundary)
        while steps[0] <= JP:
            next(gens[0], None)
            steps[0] += 1
        while steps[1] < JP:
            next(gens[1])
            steps[1] += 1
        pending = gens[1]

        # ---- remaining attention blocks with pipelined epilogues -------
        for tb in range(2, NTB):
            o_tiles[tb] = ps_o.tile([P, MS, 512], F32, tag="o", name="o_ps")
            g = block_pairs(tb, o_tiles[tb])
            next(g)                      # S(tb, 0) ahead of tb-1's last PV
            next(pending, None)          # flush PV of block tb-1
            emit_epilogue(tb - 2, o_tiles[tb - 2])
            o_tiles[tb - 2] = None
            for _ in range(JP - 1):      # steps 1..JP-1
                next(g)
            pending = g
        next(pending, None)              # flush PV of block NTB-1
        emit_epilogue(NTB - 2, o_tiles[NTB - 2])
        o_tiles[NTB - 2] = None
        emit_epilogue(NTB - 1, o_tiles[NTB - 1])
        o_tiles[NTB - 1] = None

    ctx.close()


_cache = {}


def _get_nc(n_reps: int = 1):
    if n_reps not in _cache:
        _cache[n_reps] = build(n_reps)
    return _cache[n_reps]


def _make_in_maps(inputs):
    x = np.ascontiguousarray(np.asarray(inputs["x"], dtype=np.float32))
    shared = {
        k: np.ascontiguousarray(np.asarray(inputs[k], dtype=np.float32))
        for k in ("ln_gamma", "ln_beta", "wq", "bq", "wk", "bk", "wv", "bv", "wo", "bo")
    }
    return [dict(shared, x=x[i].reshape(T, C)) for i in range(B)]


def kernel(**inputs: np.ndarray) -> np.ndarray:
    nc = _get_nc(1)
    in_maps = _make_in_maps(inputs)
    res = run_bass_kernel_spmd(nc, in_maps, list(range(B)))
    out = np.stack(
        [res.results[i]["out"].reshape(HH, WW, C) for i in range(B)], axis=0
    )
    return out.astype(np.float32)
